# revision 1
# baseline (speedup 1.0000x reference)
"""Trainium2 Bass kernel for nn_CrossMed4 (CrossMed-style GRU-over-GRU model).

Strategy (8 NeuronCores, data-parallel over the patient batch B=16 -> 2/core):
- Embedding lookups via bulk SWDGE dma_gather in token-on-partition layout.
  Tokens are pre-ordered on the host so each 128-token "rank" holds 5 groups
  x 24 codes + 8 idx-0 pads (table row 0 is the all-zero padding row).
- The sum-over-codes reduction AND the transpose into [D, group] layout happen
  in ONE PE matmul per rank against a constant 0/1 summing matrix S5.
- Monitor pair features: gathered item/value ranks multiplied elementwise on
  DVE, then rank-reduced on PE the same way.
- Both GRU levels run in transposed layout (H^T [D=128, batch*keys]) so the
  recurrence matmuls need no transposes; input-gate projections and all bias
  adds are folded into PE matmuls (rank-1 bias rows against a ones vector).
- r/z gate preactivations are accumulated directly in PSUM on top of an
  ACT-copied xg preload; sigmoid/tanh on ACT; remaining elementwise on DVE.
"""
import numpy as np

try:
    import concourse.bass as bass  # noqa: F401
except ImportError:
    import sys
    sys.path.insert(0, "/opt/trn_rl_repo")

import concourse.bacc as bacc
import concourse.bass as bass
import concourse.mybir as mybir
import concourse.tile as tile
from concourse.bass_utils import run_bass_kernel_spmd

F32 = mybir.dt.float32
I16 = mybir.dt.int16

B, V, M, L, D, OUT = 16, 16, 32, 24, 128, 193
VOCAB = {"cond": 5000, "proc": 2000, "drug": 600, "lab_item": 700,
         "lab_value": 200, "inj_item": 400, "inj_value": 200}
NCORES = 8
BL = B // NCORES            # 2 patients per core
NBV = BL * V                # 32 visit groups
TCH = 4                     # monitor steps per chunk
NCHUNK = M // TCH           # 8
GC = NBV * TCH              # 128 groups per chunk
RC = (GC + 4) // 5          # 26 ranks per chunk
IDXC = RC * 128             # 3328 idxs per chunk per stream
VRANKS = (NBV + 4) // 5     # 7
VIDX = VRANKS * 128         # 896


# --------------------------------------------------------------------------
# host-side index / weight packing
# --------------------------------------------------------------------------

def _wrap_idx(flat):
    # token i lives at [i % 16, i // 16]; the gather ucode's Q7 cores each
    # read their own 16-partition band, so replicate to all 8 bands.
    n = flat.shape[0]
    return np.tile(flat.reshape(n // 16, 16).T, (8, 1)).astype(np.int16)


def _build_monitor_idx(tok):
    """tok [BL, V, M, L] -> wrapped [128, NCHUNK*IDXC//16] int16."""
    flat = np.zeros(NCHUNK * IDXC, dtype=np.int64)
    t = np.asarray(tok)
    for c in range(NCHUNK):
        base = c * IDXC
        for r in range(RC):
            for j in range(5):
                slot = 5 * r + j
                if slot >= GC:
                    continue
                mi, rem = divmod(slot, NBV)
                b, v = divmod(rem, V)
                flat[base + r * 128 + j * 24: base + r * 128 + j * 24 + 24] = \
                    t[b, v, c * TCH + mi, :]
    return _wrap_idx(flat)


def _build_visit_idx(tok):
    flat = np.zeros(VIDX, dtype=np.int64)
    t = np.asarray(tok)
    for r in range(VRANKS):
        for j in range(5):
            slot = 5 * r + j
            if slot >= NBV:
                continue
            b, v = divmod(slot, V)
            flat[r * 128 + j * 24: r * 128 + j * 24 + 24] = t[b, v, :]
    return _wrap_idx(flat)


def _prep_shared(inputs):
    """Weight repacking shared by all cores (pure layout transforms)."""
    f = {k: np.asarray(v, dtype=np.float32) for k, v in inputs.items()
         if not k.startswith("tok_")}
    sh = {}
    mwih, mwhh = f["mgru_wih"], f["mgru_whh"]
    mbih, mbhh = f["mgru_bih"], f["mgru_bhh"]
    vwih, vwhh = f["vgru_wih"], f["vgru_whh"]
    vbih, vbhh = f["vgru_bih"], f["vgru_bhh"]

    def packT(w_keys):  # [K, 3D, D] -> [128, K*3*128], col (k*3+gi)*128+gu
        k = w_keys.shape[0]
        out = np.zeros((128, k * 3 * 128), dtype=np.float32)
        for ki in range(k):
            for gi in range(3):
                out[:, (ki * 3 + gi) * 128:(ki * 3 + gi + 1) * 128] = \
                    w_keys[ki, gi * 128:(gi + 1) * 128, :].T
        return out

    def pack_xgb(bih, bhh, keys):  # -> [1, len(keys)*384]
        rows = []
        for k in keys:
            b = bih[k].copy()
            b[:2 * D] += bhh[k][:2 * D]
            rows.append(b)
        return np.concatenate(rows)[None, :].astype(np.float32)

    sh["mwhhT"] = packT(mwhh)
    sh["mwihT34"] = packT(mwih[3:5])
    sh["mxgb34"] = pack_xgb(mbih, mbhh, [3, 4])
    sh["mwihT012"] = packT(mwih[0:3])
    sh["mxgb012"] = pack_xgb(mbih, mbhh, [0, 1, 2])
    sh["bhn_bc"] = np.repeat(mbhh[:, 2 * D:].T, NBV, axis=1).astype(np.float32)
    sh["vwhhT"] = packT(vwhh)
    sh["vwihT04"] = packT(vwih[0:5])
    sh["vxgb04"] = pack_xgb(vbih, vbhh, [0, 1, 2, 3, 4])
    u_rows, c_rows = [], []
    for k in (5, 6):
        u_rows.append(vwih[k] @ f["info_w"][k - 5])
        cv = vwih[k] @ f["info_b"][k - 5] + vbih[k]
        cv[:2 * D] += vbhh[k][:2 * D]
        c_rows.append(cv)
    sh["vxg56u"] = np.concatenate(u_rows)[None, :].astype(np.float32)
    sh["vxg56c"] = np.concatenate(c_rows)[None, :].astype(np.float32)
    sh["vbhn_bc"] = np.repeat(vbhh[:, 2 * D:].T, BL, axis=1).astype(np.float32)
    s5 = np.zeros((128, 5), dtype=np.float32)
    for j in range(5):
        s5[j * 24:(j + 1) * 24, j] = 1.0
    sh["S5"] = s5
    sh["ones"] = np.ones((1, 224), dtype=np.float32)
    fcw = np.zeros((128, 7 * OUT), dtype=np.float32)
    for k in range(7):
        fcw[:, k * OUT:(k + 1) * OUT] = f["fc_w"][k * D:(k + 1) * D, :]
    sh["fcw"] = fcw
    sh["fcb"] = f["fc_b"][None, :].astype(np.float32)
    for name in VOCAB:
        sh["emb_" + name] = f["emb_" + name]
    return sh


def _prep_core(inputs, shared, core):
    b0 = core * BL
    m = dict(shared)
    for name in ("cond", "proc", "drug"):
        m["idx_" + name] = _build_visit_idx(
            np.asarray(inputs["tok_" + name])[b0:b0 + BL])
    for name in ("lab_item", "lab_value", "inj_item", "inj_value"):
        m["idx_" + name] = _build_monitor_idx(
            np.asarray(inputs["tok_" + name])[b0:b0 + BL])
    wa = np.zeros((1, 64), dtype=np.float32)
    wa[0, :NBV] = np.asarray(inputs["weight"], np.float32)[b0:b0 + BL].reshape(NBV)
    wa[0, NBV:] = np.asarray(inputs["age"], np.float32)[b0:b0 + BL].reshape(NBV)
    m["wa"] = wa
    return m


# --------------------------------------------------------------------------
# device program
# --------------------------------------------------------------------------

def build_nc(stage="full"):
    nc = bacc.Bacc("TRN2", target_bir_lowering=False, debug=False,
                   num_devices=NCORES)
    dt = {}
    for name, voc in VOCAB.items():
        dt["emb_" + name] = nc.dram_tensor("emb_" + name, [voc, D], F32,
                                           kind="ExternalInput")
    for name in ("cond", "proc", "drug"):
        dt["idx_" + name] = nc.dram_tensor("idx_" + name, [128, VIDX // 16],
                                           I16, kind="ExternalInput")
    for name in ("lab_item", "lab_value", "inj_item", "inj_value"):
        dt["idx_" + name] = nc.dram_tensor("idx_" + name,
                                           [128, NCHUNK * IDXC // 16], I16,
                                           kind="ExternalInput")
    for name, shape in (("mwhhT", [128, 1920]), ("mwihT34", [128, 768]),
                        ("mxgb34", [1, 768]), ("mwihT012", [128, 1152]),
                        ("mxgb012", [1, 1152]), ("bhn_bc", [128, 160]),
                        ("vwhhT", [128, 2688]), ("vwihT04", [128, 1920]),
                        ("vxgb04", [1, 1920]), ("vxg56u", [1, 768]),
                        ("vxg56c", [1, 768]), ("vbhn_bc", [128, 14]),
                        ("S5", [128, 5]), ("ones", [1, 224]),
                        ("wa", [1, 64]), ("fcw", [128, 7 * OUT]),
                        ("fcb", [1, OUT])):
        dt[name] = nc.dram_tensor(name, shape, F32, kind="ExternalInput")
    out_logits = nc.dram_tensor("logits", [BL, OUT], F32, kind="ExternalOutput")

    with tile.TileContext(nc) as tc:
        _program(nc, tc, dt, out_logits, stage)
    nc.compile()
    return nc


def _program(nc, tc, dt, out_logits, stage="full"):
    STAGES = ("consts", "visit", "chunks", "gru", "vgru", "full")
    lvl = STAGES.index(stage)
    import contextlib
    ctx = contextlib.ExitStack()
    with ctx:
        cpool = ctx.enter_context(tc.tile_pool(name="const", bufs=1))
        gpool = ctx.enter_context(tc.tile_pool(name="gather", bufs=2))
        gpoolv = ctx.enter_context(tc.tile_pool(name="gatherv", bufs=1))
        spool = ctx.enter_context(tc.tile_pool(name="work", bufs=2))
        xgpool = ctx.enter_context(tc.tile_pool(name="xg34", bufs=3))
        hpool = ctx.enter_context(tc.tile_pool(name="h", bufs=2))
        ppool = ctx.enter_context(tc.tile_pool(name="psum", bufs=2,
                                               space="PSUM"))

        # ---- load constants to SBUF
        cb = {}
        for name, shape in (("mwhhT", [128, 1920]), ("mwihT34", [128, 768]),
                            ("mxgb34", [1, 768]), ("mwihT012", [128, 1152]),
                            ("mxgb012", [1, 1152]), ("bhn_bc", [128, 160]),
                            ("vwhhT", [128, 2688]), ("vwihT04", [128, 1920]),
                            ("vxgb04", [1, 1920]), ("vxg56u", [1, 768]),
                            ("vxg56c", [1, 768]), ("vbhn_bc", [128, 14]),
                            ("S5", [128, 5]), ("ones", [1, 224]),
                            ("wa", [1, 64]), ("fcw", [128, 7 * OUT]),
                            ("fcb", [1, OUT])):
            t = cpool.tile(shape, F32, tag=name)
            nc.sync.dma_start(t[:], dt[name].ap())
            cb[name] = t
        idx = {}
        for name in ("cond", "proc", "drug", "lab_item", "lab_value",
                     "inj_item", "inj_value"):
            shape = [128, VIDX // 16] if name in ("cond", "proc", "drug") \
                else [128, NCHUNK * IDXC // 16]
            t = cpool.tile(shape, I16, tag="idx_" + name)
            nc.sync.dma_start(t[:], dt["idx_" + name].ap())
            idx[name] = t

        S5 = cb["S5"]
        ones = cb["ones"]

        if lvl < 1:
            lg = spool.tile([BL, OUT], F32, tag="lg")
            nc.scalar.copy(lg[:], cb["fcw"][0:BL, 0:OUT])
            nc.sync.dma_start(out_logits.ap(), lg[:])
            return

        # ---- visit-level features eT[k] = [128, 32]
        eT = {}
        for name in ("cond", "proc", "drug"):
            gt = gpool.tile([128, VIDX], F32, tag="vgather")
            nc.gpsimd.dma_gather(
                gt[:].rearrange("p (r e) -> p r e", e=D),
                dt["emb_" + name].ap(), idx[name][:], VIDX, VIDX, D)
            pr = ppool.tile([128, VRANKS * 5], F32, tag="red")
            for r in range(VRANKS):
                nc.tensor.matmul(pr[:, 5 * r:5 * r + 5],
                                 gt[:, r * D:(r + 1) * D], S5[:],
                                 start=True, stop=True)
            et = cpool.tile([128, NBV], F32, tag="eT_" + name)
            nc.scalar.copy(et[:], pr[:, :NBV])
            eT[name] = et

        # ---- XGc for monitor keys 0-2: [128, 3*96], col gi*96 + k*32 + bv
        xgc = cpool.tile([128, 288], F32, tag="xgc")
        for k, name in enumerate(("cond", "proc", "drug")):
            pk = ppool.tile([128, 96], F32, tag="xg")
            for gi in range(3):
                off = (k * 3 + gi) * 128
                nc.tensor.matmul(pk[:, gi * 32:(gi + 1) * 32],
                                 cb["mwihT012"][:, off:off + 128],
                                 eT[name][:], start=True, stop=False)
                nc.tensor.matmul(pk[:, gi * 32:(gi + 1) * 32],
                                 cb["mxgb012"][0:1, off:off + 128],
                                 ones[0:1, 0:NBV], start=False, stop=True)
            nc.scalar.copy(
                xgc[:].rearrange("p (g c) -> p g c", g=3)[:, :, k * 32:(k + 1) * 32],
                pk[:].rearrange("p (g c) -> p g c", g=3))

        if lvl < 2:
            lg = spool.tile([BL, OUT], F32, tag="lg")
            nc.scalar.copy(lg[:], xgc[0:BL, 0:OUT])
            nc.sync.dma_start(out_logits.ap(), lg[:])
            return

        # ---- monitor chunks + GRU
        h_prev = hpool.tile([128, 160], F32, tag="H")
        nc.vector.memset(h_prev[:], 0.0)

        for c in range(NCHUNK):
            xg34 = xgpool.tile([128, 768], F32, tag="xg34")
            for k, iname, vname in ((3, "lab_item", "lab_value"),
                                    (4, "inj_item", "inj_value")):
                it = gpool.tile([128, IDXC], F32, tag=iname)
                vt = gpoolv.tile([128, IDXC], F32, tag=vname)
                # dma_gather is capped at 1024 idxs (idx-read free dim <= 64),
                # so split each chunk into <=8-rank sub-gathers.
                for tile_, name_ in ((it, iname), (vt, vname)):
                    for r0 in range(0, RC, 8):
                        nr = min(8, RC - r0)
                        nc.gpsimd.dma_gather(
                            tile_[:].rearrange("p (r e) -> p r e", e=D)
                            [:, r0:r0 + nr, :],
                            dt["emb_" + name_].ap(),
                            idx[name_][:, c * (IDXC // 16) + r0 * 8:
                                       c * (IDXC // 16) + r0 * 8 + nr * 8],
                            nr * 128, nr * 128, D)
                nc.vector.tensor_tensor(it[:], it[:], vt[:],
                                        op=mybir.AluOpType.mult)
                pr = ppool.tile([128, RC * 5], F32, tag="red")
                for r in range(RC):
                    nc.tensor.matmul(pr[:, 5 * r:5 * r + 5],
                                     it[:, r * D:(r + 1) * D], S5[:],
                                     start=True, stop=True)
                labT = spool.tile([128, GC], F32, tag="pairT")
                nc.scalar.copy(labT[:], pr[:, :GC])
                pxg = ppool.tile([128, 384], F32, tag="xg")
                for gi in range(3):
                    off = ((k - 3) * 3 + gi) * 128
                    nc.tensor.matmul(pxg[:, gi * 128:(gi + 1) * 128],
                                     cb["mwihT34"][:, off:off + 128],
                                     labT[:], start=True, stop=False)
                    nc.tensor.matmul(pxg[:, gi * 128:(gi + 1) * 128],
                                     cb["mxgb34"][0:1, off:off + 128],
                                     ones[0:1, 0:128], start=False, stop=True)
                # copy into xg34: col gi*256 + mi*64 + (k-3)*32 + bv
                for gi in range(3):
                    nc.scalar.copy(
                        xg34[:].rearrange("p (g m kb) -> p g m kb", g=3, m=TCH)
                        [:, gi, :, (k - 3) * 32:(k - 2) * 32],
                        pxg[:].rearrange("p (g m b) -> p g m b", g=3, m=TCH)
                        [:, gi, :, :])

            for mi in range(TCH if lvl >= 3 else 0):
                prz = ppool.tile([128, 320], F32, tag="prz")
                # xg preload: r/z const + per-step slices
                nc.scalar.copy(
                    prz[:].rearrange("p (g c) -> p g c", g=2)[:, :, 0:96],
                    xgc[:].rearrange("p (g c) -> p g c", g=3)[:, 0:2, :])
                nc.scalar.copy(
                    prz[:].rearrange("p (g c) -> p g c", g=2)[:, :, 96:160],
                    xg34[:].rearrange("p (g c) -> p g c", g=3)
                    [:, 0:2, mi * 64:(mi + 1) * 64])
                pn = ppool.tile([128, 160], F32, tag="pn")
                for k in range(5):
                    hs = h_prev[:, k * 32:(k + 1) * 32]
                    for gi in range(2):
                        nc.tensor.matmul(
                            prz[:, gi * 160 + k * 32: gi * 160 + (k + 1) * 32],
                            cb["mwhhT"][:, (k * 3 + gi) * 128:(k * 3 + gi + 1) * 128],
                            hs, start=False, stop=True, skip_group_check=True)
                    nc.tensor.matmul(
                        pn[:, k * 32:(k + 1) * 32],
                        cb["mwhhT"][:, (k * 3 + 2) * 128:(k * 3 + 3) * 128],
                        hs, start=True, stop=True)
                rz = spool.tile([128, 320], F32, tag="rz")
                nc.scalar.activation(rz[:], prz[:],
                                     mybir.ActivationFunctionType.Sigmoid)
                u = spool.tile([128, 160], F32, tag="u")
                nc.vector.tensor_tensor(u[:], pn[:], cb["bhn_bc"][:],
                                        op=mybir.AluOpType.add)
                nc.vector.tensor_tensor(u[:], rz[:, 0:160], u[:],
                                        op=mybir.AluOpType.mult)
                npre = spool.tile([128, 160], F32, tag="npre")
                nc.vector.tensor_tensor(
                    npre[:, 0:96], u[:, 0:96],
                    xgc[:].rearrange("p (g c) -> p g c", g=3)[:, 2, :],
                    op=mybir.AluOpType.add)
                nc.vector.tensor_tensor(
                    npre[:, 96:160], u[:, 96:160],
                    xg34[:, 2 * 256 + mi * 64: 2 * 256 + (mi + 1) * 64],
                    op=mybir.AluOpType.add)
                nt = spool.tile([128, 160], F32, tag="nt")
                nc.scalar.activation(nt[:], npre[:],
                                     mybir.ActivationFunctionType.Tanh)
                t3 = spool.tile([128, 160], F32, tag="t3")
                nc.vector.tensor_tensor(t3[:], h_prev[:], nt[:],
                                        op=mybir.AluOpType.subtract)
                nc.vector.tensor_tensor(t3[:], t3[:], rz[:, 160:320],
                                        op=mybir.AluOpType.mult)
                h_new = hpool.tile([128, 160], F32, tag="H")
                nc.vector.tensor_tensor(h_new[:], t3[:], nt[:],
                                        op=mybir.AluOpType.add)
                h_prev = h_new

        if lvl < 4:
            lg = spool.tile([BL, OUT], F32, tag="lg")
            nc.vector.memset(lg[:], 0.0)
            src = h_prev if lvl >= 3 else xg34
            nc.scalar.copy(lg[:, 0:160], src[0:BL, 0:160])
            nc.sync.dma_start(out_logits.ap(), lg[:])
            return

        # ---- visit GRU
        vxg = cpool.tile([128, 672], F32, tag="vxg")  # col gi*224 + k*32 + bv
        for k in range(5):
            pk = ppool.tile([128, 96], F32, tag="xg")
            for gi in range(3):
                off = (k * 3 + gi) * 128
                nc.tensor.matmul(pk[:, gi * 32:(gi + 1) * 32],
                                 cb["vwihT04"][:, off:off + 128],
                                 h_prev[:, k * 32:(k + 1) * 32],
                                 start=True, stop=False)
                nc.tensor.matmul(pk[:, gi * 32:(gi + 1) * 32],
                                 cb["vxgb04"][0:1, off:off + 128],
                                 ones[0:1, 0:NBV], start=False, stop=True)
            nc.scalar.copy(
                vxg[:].rearrange("p (g c) -> p g c", g=3)[:, :, k * 32:(k + 1) * 32],
                pk[:].rearrange("p (g c) -> p g c", g=3))
        for k in (5, 6):
            pk = ppool.tile([128, 96], F32, tag="xg")
            for gi in range(3):
                off = ((k - 5) * 3 + gi) * 128
                nc.tensor.matmul(pk[:, gi * 32:(gi + 1) * 32],
                                 cb["vxg56u"][0:1, off:off + 128],
                                 cb["wa"][0:1, (k - 5) * 32:(k - 4) * 32],
                                 start=True, stop=False)
                nc.tensor.matmul(pk[:, gi * 32:(gi + 1) * 32],
                                 cb["vxg56c"][0:1, off:off + 128],
                                 ones[0:1, 0:NBV], start=False, stop=True)
            nc.scalar.copy(
                vxg[:].rearrange("p (g c) -> p g c", g=3)[:, :, k * 32:(k + 1) * 32],
                pk[:].rearrange("p (g c) -> p g c", g=3))

        vh_prev = hpool.tile([128, 14], F32, tag="VH")
        nc.vector.memset(vh_prev[:], 0.0)
        for v in range(V):
            prz = ppool.tile([128, 28], F32, tag="prz")
            nc.scalar.copy(
                prz[:].rearrange("p (g c) -> p g c", g=2),
                vxg[:].rearrange("p (g k b v2) -> p g k b v2", g=3, k=7, b=BL)
                [:, 0:2, :, :, v])
            pn = ppool.tile([128, 14], F32, tag="pn")
            for k in range(7):
                hs = vh_prev[:, k * 2:(k + 1) * 2]
                for gi in range(2):
                    nc.tensor.matmul(
                        prz[:, gi * 14 + k * 2: gi * 14 + (k + 1) * 2],
                        cb["vwhhT"][:, (k * 3 + gi) * 128:(k * 3 + gi + 1) * 128],
                        hs, start=False, stop=True, skip_group_check=True)
                nc.tensor.matmul(
                    pn[:, k * 2:(k + 1) * 2],
                    cb["vwhhT"][:, (k * 3 + 2) * 128:(k * 3 + 3) * 128],
                    hs, start=True, stop=True)
            rz = spool.tile([128, 28], F32, tag="vrz")
            nc.scalar.activation(rz[:], prz[:],
                                 mybir.ActivationFunctionType.Sigmoid)
            u = spool.tile([128, 14], F32, tag="vu")
            nc.vector.tensor_tensor(u[:], pn[:], cb["vbhn_bc"][:],
                                    op=mybir.AluOpType.add)
            nc.vector.tensor_tensor(u[:], rz[:, 0:14], u[:],
                                    op=mybir.AluOpType.mult)
            nc.vector.tensor_tensor(
                u[:], u[:],
                vxg[:].rearrange("p (g k b v2) -> p g k b v2", g=3, k=7, b=BL)
                [:, 2, :, :, v],
                op=mybir.AluOpType.add)
            nt = spool.tile([128, 14], F32, tag="vnt")
            nc.scalar.activation(nt[:], u[:],
                                 mybir.ActivationFunctionType.Tanh)
            t3 = spool.tile([128, 14], F32, tag="vt3")
            nc.vector.tensor_tensor(t3[:], vh_prev[:], nt[:],
                                    op=mybir.AluOpType.subtract)
            nc.vector.tensor_tensor(t3[:], t3[:], rz[:, 14:28],
                                    op=mybir.AluOpType.mult)
            vh_new = hpool.tile([128, 14], F32, tag="VH")
            nc.vector.tensor_tensor(vh_new[:], t3[:], nt[:],
                                    op=mybir.AluOpType.add)
            vh_prev = vh_new

        # ---- FC head
        rl = spool.tile([128, 14], F32, tag="rl")
        nc.scalar.activation(rl[:], vh_prev[:],
                             mybir.ActivationFunctionType.Relu)
        pfc = ppool.tile([BL, OUT], F32, tag="pn")
        for k in range(7):
            nc.tensor.matmul(pfc[:], rl[:, k * 2:(k + 1) * 2],
                             cb["fcw"][:, k * OUT:(k + 1) * OUT],
                             start=(k == 0), stop=False)
        nc.tensor.matmul(pfc[:], ones[0:1, 0:BL], cb["fcb"][0:1, :],
                         start=False, stop=True)
        lg = spool.tile([BL, OUT], F32, tag="lg")
        nc.scalar.copy(lg[:], pfc[:])
        nc.sync.dma_start(out_logits.ap(), lg[:])


# --------------------------------------------------------------------------
# entry point
# --------------------------------------------------------------------------

_NC_CACHE = None


def kernel(**inputs):
    global _NC_CACHE
    if _NC_CACHE is None:
        _NC_CACHE = build_nc()
    nc = _NC_CACHE
    shared = _prep_shared(inputs)
    in_maps = [_prep_core(inputs, shared, c) for c in range(NCORES)]
    res = run_bass_kernel_spmd(nc, in_maps, core_ids=list(range(NCORES)))
    return np.concatenate([res.results[c]["logits"] for c in range(NCORES)],
                          axis=0).astype(np.float32)


if __name__ == "__main__":
    import reference
    inputs = {k: np.asarray(v) for k, v in reference.setup_inputs().items()}
    out = kernel(**inputs)
    print("out", out.shape, out.dtype)



# revision 12
# speedup vs baseline: 2.4075x; 2.4075x over previous
"""Trainium2 Bass kernel for nn_CrossMed4 (CrossMed-style GRU-over-GRU model).

v2 strategy (8 NeuronCores, data-parallel over the patient batch B=16 -> 2/core):
- Monitor pair features need 4 embedding streams; SWDGE dma_gather descriptor
  generation on the Q7 cores is the machine bottleneck (~7.5ns/row), so only
  the two ITEM streams are gathered. The two VALUE streams (vocab 200) are
  expanded on the PE from host-built one-hot matrices: value[d, tok] =
  tableT[vc, d]^T @ OH[vc, tok], accumulated over two 128-row vocab chunks.
- Item gathers use token order t = g*24 + l (g = mi*32 + bv chunk-local group)
  and are PE-transposed per 128-token rank into [d, tok] layout.
- Pair product on DVE (bf16), then a contiguous innermost-24 reduce gives
  labT[d, g] directly -- no reduction matmuls, no padding tokens.
- Everything on the PE is bf16 (halves instruction count vs fp32 hi/lo split
  and enables fast weight load). fp32 only in PSUM accumulation and the GRU
  master state.
- Both GRU levels run as one chain each in [d, batch*keys] layout; gate biases
  are folded into ACT-engine PSUM->SBUF copies (per-partition bias), input
  projections accumulate onto ACT-preloaded PSUM.
- Visit-level code features (cond/proc/drug) reduce+transpose in one shot:
  six accumulating matmuls with the gathered rank as stationary and a 0/1
  collapse matrix as moving.
"""
import numpy as np
import ml_dtypes

try:
    import concourse.bass as bass  # noqa: F401
except ImportError:
    import sys
    sys.path.insert(0, "/opt/trn_rl_repo")

import concourse.bacc as bacc
import concourse.bass as bass
import concourse.mybir as mybir
import concourse.tile as tile
from concourse.bass_utils import run_bass_kernel_spmd

F32 = mybir.dt.float32
BF16 = mybir.dt.float16
I16 = mybir.dt.int16
BF = np.float16

B, V, M, L, D, OUT = 16, 16, 32, 24, 128, 193
VOCAB = {"cond": 5000, "proc": 2000, "drug": 600, "lab_item": 700,
         "lab_value": 200, "inj_item": 400, "inj_value": 200}
NCORES = 8
BL = B // NCORES            # 2 patients per core
NBV = BL * V                # 32 visit groups
TCH = 4                     # monitor steps per chunk
NCHUNK = M // TCH           # 8
GC = NBV * TCH              # 128 groups per chunk (= mi*32 + bv)
TOKC = GC * L               # 3072 tokens per chunk per stream
NRK = TOKC // 128           # 24 ranks per chunk per stream
VIDX = NBV * L              # 768 visit-stream tokens (6 ranks)
AF = mybir.ActivationFunctionType


# --------------------------------------------------------------------------
# host-side packing
# --------------------------------------------------------------------------

def _wrap_idx(flat):
    # token i lives at [i % 16, i // 16]; the gather ucode's Q7 cores each
    # read their own 16-partition band, so replicate to all 8 bands.
    n = flat.shape[0]
    return np.tile(flat.reshape(n // 16, 16).T, (8, 1)).astype(np.int16)


def _packT(w_keys):  # [K, 3D, D] -> [128, K*3*128] bf16, col (k*3+gi)*128+d'
    k = w_keys.shape[0]
    out = np.zeros((128, k * 3 * 128), dtype=np.float32)
    for ki in range(k):
        for gi in range(3):
            out[:, (ki * 3 + gi) * 128:(ki * 3 + gi + 1) * 128] = \
                w_keys[ki, gi * 128:(gi + 1) * 128, :].T
    return out.astype(BF)


def _bias_cols(bih, bhh, keys):  # [128, len(keys)*3] f32, col k*3+gi
    cols = []
    for k in keys:
        for gi in range(3):
            b = bih[k][gi * D:(gi + 1) * D].copy()
            if gi < 2:
                b += bhh[k][gi * D:(gi + 1) * D]
            cols.append(b)
    return np.stack(cols, axis=1).astype(np.float32)


def _prep_shared(inputs):
    f = {k: np.asarray(v, dtype=np.float32) for k, v in inputs.items()
         if not k.startswith("tok_")}
    sh = {}
    # gatherable tables, bf16
    for name in ("cond", "proc", "drug", "lab_item", "inj_item"):
        sh["emb_" + name] = f["emb_" + name].astype(BF)
    # value tables packed for one-hot matmuls: [128, 2*128], vtp[p, c*128+d]
    # = T[c*128+p, d]
    for name in ("lab_value", "inj_value"):
        pad = np.zeros((256, D), dtype=np.float32)
        pad[:VOCAB[name]] = f["emb_" + name]
        sh["vt_" + name[:3]] = np.ascontiguousarray(
            pad.reshape(2, 128, D).transpose(1, 0, 2).reshape(128, 256)
        ).astype(BF)

    mwih, mwhh = f["mgru_wih"], f["mgru_whh"]
    mbih, mbhh = f["mgru_bih"], f["mgru_bhh"]
    vwih, vwhh = f["vgru_wih"], f["vgru_whh"]
    vbih, vbhh = f["vgru_bih"], f["vgru_bhh"]

    sh["mwhhT"] = _packT(mwhh)                   # [128, 1920]
    sh["mwihT012"] = _packT(mwih[0:3])           # [128, 1152]
    sh["mwihT34"] = _packT(mwih[3:5])            # [128, 768]
    sh["vwhhT"] = _packT(vwhh)                   # [128, 2688]
    sh["vwihT04"] = _packT(vwih[0:5])            # [128, 1920]
    sh["mb012c"] = _bias_cols(mbih, mbhh, [0, 1, 2])   # [128, 9]
    sh["mb34c"] = _bias_cols(mbih, mbhh, [3, 4])       # [128, 6]
    sh["vb04c"] = _bias_cols(vbih, vbhh, [0, 1, 2, 3, 4])  # [128, 15]
    sh["mbhn_bc"] = np.repeat(mbhh[:, 2 * D:].T, NBV, axis=1).astype(np.float32)
    sh["vbhn_bc"] = np.repeat(vbhh[:, 2 * D:].T, BL, axis=1).astype(np.float32)
    # keys 5,6 (weight/age): xg = u_k * wa[bv] + c_k  (rank-1)
    u_rows, c_cols = [], []
    for k in (5, 6):
        u_rows.append(vwih[k] @ f["info_w"][k - 5])
        cv = vwih[k] @ f["info_b"][k - 5] + vbih[k]
        cv[:2 * D] += vbhh[k][:2 * D]
        for gi in range(3):
            c_cols.append(cv[gi * D:(gi + 1) * D])
    sh["vxg56u"] = np.concatenate(u_rows)[None, :].astype(BF)   # [1, 768]
    sh["vb56c"] = np.stack(c_cols, axis=1).astype(np.float32)   # [128, 6]
    # collapse matrix for visit streams: S4[bv*4+cb, bv] = 1
    s4 = np.zeros((128, NBV), dtype=np.float32)
    for bv in range(NBV):
        s4[bv * 4:(bv + 1) * 4, bv] = 1.0
    sh["S4"] = s4.astype(BF)
    sh["I128"] = np.eye(128, dtype=np.float32).astype(BF)
    fcw = np.zeros((128, 7 * OUT), dtype=np.float32)
    for k in range(7):
        fcw[:, k * OUT:(k + 1) * OUT] = f["fc_w"][k * D:(k + 1) * D, :]
    sh["fcw"] = fcw.astype(BF)
    sh["fcb2"] = np.tile(f["fc_b"][None, :], (BL, 1)).astype(np.float32)
    return sh


def _mon_flat(tok):
    """tok [BL,V,M,L] int -> flat [NCHUNK*TOKC] with order
    flat[c*TOKC + (mi*NBV + b*V + v)*L + l] = tok[b, v, c*TCH+mi, l]."""
    t = np.asarray(tok).reshape(BL, V, NCHUNK, TCH, L)
    return np.ascontiguousarray(t.transpose(2, 3, 0, 1, 4)).reshape(-1)


def _prep_core(inputs, shared, core):
    b0 = core * BL
    m = dict(shared)
    # item gather indices
    for name in ("lab_item", "inj_item"):
        flat = _mon_flat(np.asarray(inputs["tok_" + name])[b0:b0 + BL])
        m["idx_" + name] = _wrap_idx(flat)
    # value one-hot matrices [128, NCHUNK*2*TOKC] bf16
    for name, tag in (("lab_value", "oh_lab"), ("inj_value", "oh_inj")):
        flat = _mon_flat(np.asarray(inputs["tok_" + name])[b0:b0 + BL])
        toks = flat.reshape(NCHUNK, TOKC)
        oh = np.zeros((128, NCHUNK, 2, TOKC), dtype=BF)
        prow = np.arange(128)[:, None]
        for c in range(NCHUNK):
            for vc in range(2):
                oh[:, c, vc, :] = (toks[c][None, :] == vc * 128 + prow)
        m[tag] = np.ascontiguousarray(oh).reshape(128, NCHUNK * 2 * TOKC)
    # visit-stream indices: flat[r*128 + (b*V+v)*4 + cb] = tok[b,v,cb*6+r]
    for name in ("cond", "proc", "drug"):
        t = np.asarray(inputs["tok_" + name])[b0:b0 + BL].reshape(BL, V, 4, 6)
        flat = np.ascontiguousarray(t.transpose(3, 0, 1, 2)).reshape(-1)
        m["idx_" + name] = _wrap_idx(flat)
    wa = np.zeros((1, 64), dtype=np.float32)
    wa[0, :NBV] = np.asarray(inputs["weight"], np.float32)[b0:b0 + BL].reshape(NBV)
    wa[0, NBV:] = np.asarray(inputs["age"], np.float32)[b0:b0 + BL].reshape(NBV)
    m["wa"] = wa.astype(BF)
    return m


# --------------------------------------------------------------------------
# device program
# --------------------------------------------------------------------------

CONSTS = (("mwhhT", [128, 1920], BF16), ("mwihT012", [128, 1152], BF16),
          ("mwihT34", [128, 768], BF16), ("vwhhT", [128, 2688], BF16),
          ("vwihT04", [128, 1920], BF16), ("mb012c", [128, 9], F32),
          ("mb34c", [128, 6], F32), ("vb04c", [128, 15], F32),
          ("mbhn_bc", [128, 160], F32), ("vbhn_bc", [128, 14], F32),
          ("vxg56u", [1, 768], BF16), ("vb56c", [128, 6], F32),
          ("S4", [128, NBV], BF16), ("I128", [128, 128], BF16),
          ("fcw", [128, 7 * OUT], BF16), ("fcb2", [BL, OUT], F32),
          ("wa", [1, 64], BF16), ("vt_lab", [128, 256], BF16),
          ("vt_inj", [128, 256], BF16))


def build_nc(debug=False):
    nc = bacc.Bacc("TRN2", target_bir_lowering=False, debug=False,
                   num_devices=NCORES)
    dt = {}
    for name in ("cond", "proc", "drug", "lab_item", "inj_item"):
        dt["emb_" + name] = nc.dram_tensor("emb_" + name, [VOCAB[name], D],
                                           BF16, kind="ExternalInput")
    for name in ("lab_item", "inj_item"):
        dt["idx_" + name] = nc.dram_tensor("idx_" + name,
                                           [128, NCHUNK * TOKC // 16], I16,
                                           kind="ExternalInput")
    for name in ("cond", "proc", "drug"):
        dt["idx_" + name] = nc.dram_tensor("idx_" + name, [128, VIDX // 16],
                                           I16, kind="ExternalInput")
    for name in ("oh_lab", "oh_inj"):
        dt[name] = nc.dram_tensor(name, [128, NCHUNK * 2 * TOKC], BF16,
                                  kind="ExternalInput")
    for name, shape, dty in CONSTS:
        dt[name] = nc.dram_tensor(name, shape, dty, kind="ExternalInput")
    out_logits = nc.dram_tensor("logits", [BL, OUT], F32, kind="ExternalOutput")
    if debug:
        dt["dbg_eT"] = nc.dram_tensor("dbg_eT", [128, NBV], F32,
                                      kind="ExternalOutput")
        dt["dbg_labT"] = nc.dram_tensor("dbg_labT", [128, GC], F32,
                                        kind="ExternalOutput")
        dt["dbg_xg3"] = nc.dram_tensor("dbg_xg3", [128, 3 * GC], F32,
                                       kind="ExternalOutput")
        dt["dbg_h1"] = nc.dram_tensor("dbg_h1", [128, 160], F32,
                                      kind="ExternalOutput")
        dt["dbg_hfin"] = nc.dram_tensor("dbg_hfin", [128, 160], F32,
                                        kind="ExternalOutput")
        dt["dbg_vxg"] = nc.dram_tensor("dbg_vxg", [128, 672], F32,
                                       kind="ExternalOutput")
        dt["dbg_xgc"] = nc.dram_tensor("dbg_xgc", [128, 288], F32,
                                       kind="ExternalOutput")
        dt["dbg_prz"] = nc.dram_tensor("dbg_prz", [128, 480], F32,
                                       kind="ExternalOutput")
        dt["dbg_hb0"] = nc.dram_tensor("dbg_hb0", [128, 160], F32,
                                       kind="ExternalOutput")

    with tile.TileContext(nc) as tc:
        _program(nc, tc, dt, out_logits, debug)
    nc.compile()
    return nc


def _program(nc, tc, dt, out_logits, debug=False):
    import contextlib
    ctx = contextlib.ExitStack()
    with ctx:
        cpool = ctx.enter_context(tc.tile_pool(name="const", bufs=1))
        vgpool = ctx.enter_context(tc.tile_pool(name="vgather", bufs=1))
        gpool = ctx.enter_context(tc.tile_pool(name="gather", bufs=2))
        ohpool = ctx.enter_context(tc.tile_pool(name="oh", bufs=2))
        spool = ctx.enter_context(tc.tile_pool(name="work", bufs=2))
        xgpool = ctx.enter_context(tc.tile_pool(name="xg", bufs=2))
        hpool = ctx.enter_context(tc.tile_pool(name="h", bufs=2))
        ppool = ctx.enter_context(tc.tile_pool(name="psum", bufs=2,
                                               space="PSUM"))

        cb = {}
        for name, shape, dty in CONSTS:
            t = cpool.tile(shape, dty, tag=name, name=name)
            nc.sync.dma_start(t[:], dt[name].ap())
            cb[name] = t
        idx = {}
        for name in ("cond", "proc", "drug", "lab_item", "inj_item"):
            shape = [128, VIDX // 16] if name in ("cond", "proc", "drug") \
                else [128, NCHUNK * TOKC // 16]
            t = cpool.tile(shape, I16, tag="idx_" + name, name="idx_" + name)
            nc.sync.dma_start(t[:], dt["idx_" + name].ap())
            idx[name] = t

        I128 = cb["I128"]

        # ---- visit-level features eT[k] = [128 d, 32 bv] (bf16)
        eTb = {}
        for name in ("cond", "proc", "drug"):
            vG = vgpool.tile([128, VIDX], BF16, tag="vG_" + name, name="vG")
            nc.gpsimd.dma_gather(
                vG[:].rearrange("p (r e) -> p r e", e=D),
                dt["emb_" + name].ap(), idx[name][:], VIDX, VIDX, D)
            eTp = ppool.tile([128, NBV], F32, tag="pxg", name="eTp")
            for r in range(6):
                nc.tensor.matmul(eTp[:],
                                 vG[:].rearrange("p (r e) -> p r e", e=D)[:, r, :],
                                 cb["S4"][:], start=(r == 0), stop=(r == 5))
            et = cpool.tile([128, NBV], BF16, tag="eT_" + name, name="eT")
            nc.scalar.copy(et[:], eTp[:])
            if debug and name == "proc":
                etf = cpool.tile([128, NBV], F32, tag="etf", name="etf")
                nc.scalar.copy(etf[:], eTp[:])
                nc.sync.dma_start(dt["dbg_eT"].ap(), etf[:])
            eTb[name] = et

        # ---- xgc for monitor keys 0-2: [128, 3, 96] (gi, k*32+bv), bias folded
        xgc = cpool.tile([128, 3, 96], F32, tag="xgc", name="xgc")
        for k, name in enumerate(("cond", "proc", "drug")):
            pk = ppool.tile([128, 96], F32, tag="pxg", name="pk")
            for gi in range(3):
                nc.tensor.matmul(pk[:, gi * 32:(gi + 1) * 32],
                                 cb["mwihT012"][:, (k * 3 + gi) * 128:
                                                (k * 3 + gi + 1) * 128],
                                 eTb[name][:], start=True, stop=True)
            for gi in range(3):
                nc.scalar.activation(xgc[:, gi, k * 32:(k + 1) * 32],
                                     pk[:, gi * 32:(gi + 1) * 32], AF.Identity,
                                     bias=cb["mb012c"][:, k * 3 + gi:
                                                       k * 3 + gi + 1])

        if debug:
            nc.sync.dma_start(dt["dbg_xgc"].ap(),
                              xgc[:].rearrange("p a b -> p (a b)"))

        # ---- visit xg for keys 5,6 (weight/age), rank-1 + const
        vxg = cpool.tile([128, 3, 7, NBV], F32, tag="vxg", name="vxg")
        for k in (5, 6):
            p56 = ppool.tile([128, 96], F32, tag="pxg", name="p56")
            for gi in range(3):
                nc.tensor.matmul(p56[:, gi * 32:(gi + 1) * 32],
                                 cb["vxg56u"][0:1, ((k - 5) * 3 + gi) * 128:
                                              ((k - 5) * 3 + gi + 1) * 128],
                                 cb["wa"][0:1, (k - 5) * 32:(k - 4) * 32],
                                 start=True, stop=True)
            for gi in range(3):
                nc.scalar.activation(vxg[:, gi, k, :],
                                     p56[:, gi * 32:(gi + 1) * 32], AF.Identity,
                                     bias=cb["vb56c"][:, (k - 5) * 3 + gi:
                                                      (k - 5) * 3 + gi + 1])

        # ---- monitor chunks + GRU chain
        h = hpool.tile([128, 160], F32, tag="h", name="h0")
        hb = hpool.tile([128, 160], BF16, tag="hb", name="hb0")
        nc.vector.memset(h[:], 0.0)
        nc.vector.memset(hb[:], 0.0)

        for c in range(NCHUNK):
            xgk = {}
            for k, iname, ohname, vtname in (
                    (3, "lab_item", "oh_lab", "vt_lab"),
                    (4, "inj_item", "oh_inj", "vt_inj")):
                iG = gpool.tile([128, TOKC], BF16, tag="iG" + iname, name="iG")
                iGr = iG[:].rearrange("p (r e) -> p r e", e=D)
                for s in range(3):
                    nc.gpsimd.dma_gather(
                        iGr[:, s * 8:(s + 1) * 8, :],
                        dt["emb_" + iname].ap(),
                        idx[iname][:, c * (TOKC // 16) + s * 64:
                                   c * (TOKC // 16) + (s + 1) * 64],
                        1024, 1024, D)
                ohT = ohpool.tile([128, 2, TOKC], BF16, tag=ohname, name="ohT")
                nc.sync.dma_start(
                    ohT[:], dt[ohname].ap()
                    .rearrange("p (c x) -> p c x", c=NCHUNK)[:, c, :]
                    .rearrange("p (v t) -> p v t", v=2))
                prod = spool.tile([128, TOKC], BF16, tag="prod" + iname,
                                  name="prod")
                for blk in range(6):
                    tP = ppool.tile([128, 512], BF16, tag="tP", name="tP")
                    for rr in range(4):
                        nc.tensor.transpose(tP[:, rr * 128:(rr + 1) * 128],
                                            iGr[:, blk * 4 + rr, :], I128[:])
                    valP = ppool.tile([128, 512], F32, tag="valP", name="valP")
                    for vc in range(2):
                        nc.tensor.matmul(valP[:],
                                         cb[vtname][:, vc * 128:(vc + 1) * 128],
                                         ohT[:, vc, blk * 512:(blk + 1) * 512],
                                         start=(vc == 0), stop=(vc == 1))
                    valS = spool.tile([128, 512], BF16, tag="valS" + iname,
                                      name="valS")
                    nc.scalar.copy(valS[:], valP[:])
                    nc.vector.tensor_tensor(prod[:, blk * 512:(blk + 1) * 512],
                                            tP[:], valS[:],
                                            op=mybir.AluOpType.mult)
                red = spool.tile([128, GC], F32, tag="red" + iname, name="red")
                nc.vector.tensor_reduce(
                    red[:], prod[:].rearrange("p (g l) -> p g l", l=L),
                    axis=mybir.AxisListType.X, op=mybir.AluOpType.add)
                redb = spool.tile([128, GC], BF16, tag="redb" + iname,
                                  name="redb")
                nc.scalar.copy(redb[:], red[:])
                pxg = ppool.tile([128, 384], F32, tag="pxg", name="pxg")
                for gi in range(3):
                    nc.tensor.matmul(pxg[:, gi * 128:(gi + 1) * 128],
                                     cb["mwihT34"][:, ((k - 3) * 3 + gi) * 128:
                                                   ((k - 3) * 3 + gi + 1) * 128],
                                     redb[:], start=True, stop=True)
                if debug and c == 0 and k == 3:
                    nc.sync.dma_start(dt["dbg_labT"].ap(), red[:])
                xg = xgpool.tile([128, 3, TCH, NBV], F32, tag=f"xg{k}",
                                 name="xg")
                for gi in range(3):
                    nc.scalar.activation(
                        xg[:, gi, :, :],
                        pxg[:, gi * 128:(gi + 1) * 128]
                        .rearrange("p (m b) -> p m b", m=TCH), AF.Identity,
                        bias=cb["mb34c"][:, (k - 3) * 3 + gi:
                                         (k - 3) * 3 + gi + 1])
                if debug and c == 0 and k == 3:
                    nc.sync.dma_start(
                        dt["dbg_xg3"].ap(),
                        xg[:].rearrange("p a b c2 -> p (a b c2)"))
                xgk[k] = xg

            for mi in range(TCH):
                przt = ppool.tile([128, 320], F32, tag="prz", name="przt",
                                  bufs=1)
                prz = przt[:].rearrange("p (g x) -> p g x", g=2)
                pn = ppool.tile([128, 160], F32, tag="pn", name="pn", bufs=1)
                nc.scalar.copy(prz[:, :, 0:96], xgc[:, 0:2, :])
                nc.scalar.copy(prz[:, :, 96:128], xgk[3][:, 0:2, mi, :])
                nc.scalar.copy(prz[:, :, 128:160], xgk[4][:, 0:2, mi, :])
                for k in range(5):
                    hs = hb[:, k * 32:(k + 1) * 32]
                    for gi in range(2):
                        nc.tensor.matmul(
                            prz[:, gi, k * 32:(k + 1) * 32],
                            cb["mwhhT"][:, (k * 3 + gi) * 128:
                                        (k * 3 + gi + 1) * 128],
                            hs, start=False, stop=True, skip_group_check=True)
                    nc.tensor.matmul(
                        pn[:, k * 32:(k + 1) * 32],
                        cb["mwhhT"][:, (k * 3 + 2) * 128:(k * 3 + 3) * 128],
                        hs, start=True, stop=True)
                rz = spool.tile([128, 320], F32, tag="rz", name="rz")
                nc.scalar.activation(rz[:], przt[:], AF.Sigmoid)
                u = spool.tile([128, 160], F32, tag="u", name="u")
                nc.vector.tensor_tensor(u[:], pn[:], cb["mbhn_bc"][:],
                                        op=mybir.AluOpType.add)
                nc.vector.tensor_tensor(u[:], rz[:, 0:160], u[:],
                                        op=mybir.AluOpType.mult)
                npre = spool.tile([128, 160], F32, tag="npre", name="npre")
                nc.vector.tensor_tensor(npre[:, 0:96], u[:, 0:96],
                                        xgc[:, 2, :], op=mybir.AluOpType.add)
                nc.vector.tensor_tensor(npre[:, 96:128], u[:, 96:128],
                                        xgk[3][:, 2, mi, :],
                                        op=mybir.AluOpType.add)
                nc.vector.tensor_tensor(npre[:, 128:160], u[:, 128:160],
                                        xgk[4][:, 2, mi, :],
                                        op=mybir.AluOpType.add)
                nt = spool.tile([128, 160], F32, tag="nt", name="nt")
                nc.scalar.activation(nt[:], npre[:], AF.Tanh)
                t3 = spool.tile([128, 160], F32, tag="t3", name="t3")
                nc.vector.tensor_tensor(t3[:], h[:], nt[:],
                                        op=mybir.AluOpType.subtract)
                nc.vector.tensor_tensor(t3[:], t3[:], rz[:, 160:320],
                                        op=mybir.AluOpType.mult)
                h = hpool.tile([128, 160], F32, tag="h", name="h")
                nc.vector.tensor_tensor(h[:], t3[:], nt[:],
                                        op=mybir.AluOpType.add)
                hb = hpool.tile([128, 160], BF16, tag="hb", name="hb")
                nc.scalar.copy(hb[:], h[:])
                if debug and c == 0 and mi == 0:
                    nc.sync.dma_start(dt["dbg_h1"].ap(), h[:])

        if debug:
            nc.sync.dma_start(dt["dbg_hfin"].ap(), h[:])

        # ---- visit xg for keys 0-4 from monitor hidden state
        for k in range(5):
            pk = ppool.tile([128, 96], F32, tag="pxg", name="pkv")
            for gi in range(3):
                nc.tensor.matmul(pk[:, gi * 32:(gi + 1) * 32],
                                 cb["vwihT04"][:, (k * 3 + gi) * 128:
                                               (k * 3 + gi + 1) * 128],
                                 hb[:, k * 32:(k + 1) * 32],
                                 start=True, stop=True)
            for gi in range(3):
                nc.scalar.activation(vxg[:, gi, k, :],
                                     pk[:, gi * 32:(gi + 1) * 32], AF.Identity,
                                     bias=cb["vb04c"][:, k * 3 + gi:
                                                      k * 3 + gi + 1])

        if debug:
            nc.sync.dma_start(dt["dbg_vxg"].ap(),
                              vxg[:].rearrange("p a b c2 -> p (a b c2)"))

        # ---- visit GRU chain (7 keys, 16 steps, batch BL=2 per key)
        vxgv = vxg[:].rearrange("p g k (b v2) -> p g k b v2", b=BL)
        vh = hpool.tile([128, 14], F32, tag="vh", name="vh0")
        vhb = hpool.tile([128, 14], BF16, tag="vhb", name="vhb0")
        nc.vector.memset(vh[:], 0.0)
        nc.vector.memset(vhb[:], 0.0)
        for v in range(V):
            pvrzt = ppool.tile([128, 28], F32, tag="prz", name="pvrzt",
                               bufs=1)
            pvrz = pvrzt[:].rearrange("p (g x) -> p g x", g=2)
            pvn = ppool.tile([128, 14], F32, tag="pn", name="pvn", bufs=1)
            nc.scalar.copy(pvrz[:, :, :], vxgv[:, 0:2, :, :, v])
            for k in range(7):
                hs = vhb[:, k * 2:(k + 1) * 2]
                for gi in range(2):
                    nc.tensor.matmul(
                        pvrz[:, gi, k * 2:(k + 1) * 2],
                        cb["vwhhT"][:, (k * 3 + gi) * 128:
                                    (k * 3 + gi + 1) * 128],
                        hs, start=False, stop=True, skip_group_check=True)
                nc.tensor.matmul(
                    pvn[:, k * 2:(k + 1) * 2],
                    cb["vwhhT"][:, (k * 3 + 2) * 128:(k * 3 + 3) * 128],
                    hs, start=True, stop=True)
            vrz = spool.tile([128, 28], F32, tag="vrz", name="vrz")
            nc.scalar.activation(vrz[:], pvrzt[:], AF.Sigmoid)
            vu = spool.tile([128, 14], F32, tag="vu", name="vu")
            nc.vector.tensor_tensor(vu[:], pvn[:], cb["vbhn_bc"][:],
                                    op=mybir.AluOpType.add)
            nc.vector.tensor_tensor(vu[:], vrz[:, 0:14], vu[:],
                                    op=mybir.AluOpType.mult)
            nc.vector.tensor_tensor(vu[:], vu[:], vxgv[:, 2, :, :, v],
                                    op=mybir.AluOpType.add)
            vnt = spool.tile([128, 14], F32, tag="vnt", name="vnt")
            nc.scalar.activation(vnt[:], vu[:], AF.Tanh)
            vt3 = spool.tile([128, 14], F32, tag="vt3", name="vt3")
            nc.vector.tensor_tensor(vt3[:], vh[:], vnt[:],
                                    op=mybir.AluOpType.subtract)
            nc.vector.tensor_tensor(vt3[:], vt3[:], vrz[:, 14:28],
                                    op=mybir.AluOpType.mult)
            vh = hpool.tile([128, 14], F32, tag="vh", name="vh")
            nc.vector.tensor_tensor(vh[:], vt3[:], vnt[:],
                                    op=mybir.AluOpType.add)
            vhb = hpool.tile([128, 14], BF16, tag="vhb", name="vhb")
            nc.scalar.copy(vhb[:], vh[:])

        # ---- FC head
        rlb = spool.tile([128, 14], BF16, tag="rlb", name="rlb")
        nc.scalar.activation(rlb[:], vh[:], AF.Relu)
        pfc = ppool.tile([BL, OUT], F32, tag="pn", name="pfc", bufs=1)
        for k in range(7):
            nc.tensor.matmul(pfc[:], rlb[:, k * 2:(k + 1) * 2],
                             cb["fcw"][:, k * OUT:(k + 1) * OUT],
                             start=(k == 0), stop=(k == 6))
        lg = spool.tile([BL, OUT], F32, tag="lg", name="lg")
        nc.vector.tensor_tensor(lg[:], pfc[:], cb["fcb2"][:],
                                op=mybir.AluOpType.add)
        nc.sync.dma_start(out_logits.ap(), lg[:])


# --------------------------------------------------------------------------
# entry point
# --------------------------------------------------------------------------

_NC_CACHE = None


def kernel(**inputs):
    global _NC_CACHE
    if _NC_CACHE is None:
        _NC_CACHE = build_nc()
    nc = _NC_CACHE
    shared = _prep_shared(inputs)
    in_maps = [_prep_core(inputs, shared, c) for c in range(NCORES)]
    res = run_bass_kernel_spmd(nc, in_maps, core_ids=list(range(NCORES)))
    return np.concatenate([res.results[c]["logits"] for c in range(NCORES)],
                          axis=0).astype(np.float32)


if __name__ == "__main__":
    import reference
    inputs = {k: np.asarray(v) for k, v in reference.setup_inputs().items()}
    out = kernel(**inputs)
    print("out", out.shape, out.dtype)


# revision 16
# speedup vs baseline: 4.3186x; 1.7939x over previous
"""Trainium2 Bass kernel for nn_CrossMed4 (CrossMed-style GRU-over-GRU model).

v2 strategy (8 NeuronCores, data-parallel over the patient batch B=16 -> 2/core):
- Monitor pair features need 4 embedding streams; SWDGE dma_gather descriptor
  generation on the Q7 cores is the machine bottleneck (~7.5ns/row), so only
  the two ITEM streams are gathered. The two VALUE streams (vocab 200) are
  expanded on the PE from host-built one-hot matrices: value[d, tok] =
  tableT[vc, d]^T @ OH[vc, tok], accumulated over two 128-row vocab chunks.
- Item gathers use token order t = g*24 + l (g = mi*32 + bv chunk-local group)
  and are PE-transposed per 128-token rank into [d, tok] layout.
- Pair product on DVE (bf16), then a contiguous innermost-24 reduce gives
  labT[d, g] directly -- no reduction matmuls, no padding tokens.
- Everything on the PE is bf16 (halves instruction count vs fp32 hi/lo split
  and enables fast weight load). fp32 only in PSUM accumulation and the GRU
  master state.
- Both GRU levels run as one chain each in [d, batch*keys] layout; gate biases
  are folded into ACT-engine PSUM->SBUF copies (per-partition bias), input
  projections accumulate onto ACT-preloaded PSUM.
- Visit-level code features (cond/proc/drug) reduce+transpose in one shot:
  six accumulating matmuls with the gathered rank as stationary and a 0/1
  collapse matrix as moving.
"""
import numpy as np
import ml_dtypes

try:
    import concourse.bass as bass  # noqa: F401
except ImportError:
    import sys
    sys.path.insert(0, "/opt/trn_rl_repo")

import concourse.bacc as bacc
import concourse.bass as bass
import concourse.mybir as mybir
import concourse.tile as tile
from concourse.bass_utils import run_bass_kernel_spmd

F32 = mybir.dt.float32
BF16 = mybir.dt.float16
I16 = mybir.dt.int16
BF = np.float16

B, V, M, L, D, OUT = 16, 16, 32, 24, 128, 193
VOCAB = {"cond": 5000, "proc": 2000, "drug": 600, "lab_item": 700,
         "lab_value": 200, "inj_item": 400, "inj_value": 200}
NCORES = 8
BL = B // NCORES            # 2 patients per core
NBV = BL * V                # 32 visit groups
TCH = 4                     # monitor steps per chunk
NCHUNK = M // TCH           # 8
GC = NBV * TCH              # 128 groups per chunk (= mi*32 + bv)
TOKC = GC * L               # 3072 tokens per chunk per stream
NRK = TOKC // 128           # 24 ranks per chunk per stream
VIDX = NBV * L              # 768 visit-stream tokens (6 ranks)
AF = mybir.ActivationFunctionType


# --------------------------------------------------------------------------
# host-side packing
# --------------------------------------------------------------------------

def _wrap_idx(flat):
    # token i lives at [i % 16, i // 16]; the gather ucode's Q7 cores each
    # read their own 16-partition band, so replicate to all 8 bands.
    n = flat.shape[0]
    return np.tile(flat.reshape(n // 16, 16).T, (8, 1)).astype(np.int16)


def _packT(w_keys):  # [K, 3D, D] -> [128, K*3*128] bf16, col (k*3+gi)*128+d'
    k = w_keys.shape[0]
    out = np.zeros((128, k * 3 * 128), dtype=np.float32)
    for ki in range(k):
        for gi in range(3):
            out[:, (ki * 3 + gi) * 128:(ki * 3 + gi + 1) * 128] = \
                w_keys[ki, gi * 128:(gi + 1) * 128, :].T
    return out.astype(BF)


def _bias_cols(bih, bhh, keys):  # [128, len(keys)*3] f32, col k*3+gi
    cols = []
    for k in keys:
        for gi in range(3):
            b = bih[k][gi * D:(gi + 1) * D].copy()
            if gi < 2:
                b += bhh[k][gi * D:(gi + 1) * D]
            cols.append(b)
    return np.stack(cols, axis=1).astype(np.float32)


def _prep_shared(inputs):
    f = {k: np.asarray(v, dtype=np.float32) for k, v in inputs.items()
         if not k.startswith("tok_")}
    sh = {}
    # gatherable tables, fp16
    for name in ("cond", "proc", "drug", "lab_item"):
        sh["emb_" + name] = f["emb_" + name].astype(BF)
    # tables packed for one-hot matmuls: [128, nvc*128], vtp[p, c*128+d]
    # = T[c*128+p, d]
    for name, tag, nvc in (("lab_value", "vt_lab", 2), ("inj_value", "vt_inj", 2),
                           ("inj_item", "vt_inji", 4)):
        pad = np.zeros((nvc * 128, D), dtype=np.float32)
        pad[:VOCAB[name]] = f["emb_" + name]
        sh[tag] = np.ascontiguousarray(
            pad.reshape(nvc, 128, D).transpose(1, 0, 2).reshape(128, nvc * 128)
        ).astype(BF)

    mwih, mwhh = f["mgru_wih"], f["mgru_whh"]
    mbih, mbhh = f["mgru_bih"], f["mgru_bhh"]
    vwih, vwhh = f["vgru_wih"], f["vgru_whh"]
    vbih, vbhh = f["vgru_bih"], f["vgru_bhh"]

    sh["mwhhT"] = _packT(mwhh)                   # [128, 1920]
    sh["mwihT012"] = _packT(mwih[0:3])           # [128, 1152]
    sh["mwihT34"] = _packT(mwih[3:5])            # [128, 768]
    sh["vwhhT"] = _packT(vwhh)                   # [128, 2688]
    sh["vwihT04"] = _packT(vwih[0:5])            # [128, 1920]
    sh["mb012c"] = _bias_cols(mbih, mbhh, [0, 1, 2])   # [128, 9]
    sh["mb34c"] = _bias_cols(mbih, mbhh, [3, 4])       # [128, 6]
    sh["vb04c"] = _bias_cols(vbih, vbhh, [0, 1, 2, 3, 4])  # [128, 15]
    sh["mbhn_bc"] = np.repeat(mbhh[:, 2 * D:].T, NBV, axis=1).astype(np.float32)
    sh["vbhn_bc"] = np.repeat(vbhh[:, 2 * D:].T, BL, axis=1).astype(np.float32)
    # keys 5,6 (weight/age): xg = u_k * wa[bv] + c_k  (rank-1)
    u_rows, c_cols = [], []
    for k in (5, 6):
        u_rows.append(vwih[k] @ f["info_w"][k - 5])
        cv = vwih[k] @ f["info_b"][k - 5] + vbih[k]
        cv[:2 * D] += vbhh[k][:2 * D]
        for gi in range(3):
            c_cols.append(cv[gi * D:(gi + 1) * D])
    sh["vxg56u"] = np.concatenate(u_rows)[None, :].astype(BF)   # [1, 768]
    sh["vb56c"] = np.stack(c_cols, axis=1).astype(np.float32)   # [128, 6]
    # collapse matrix for visit streams: S4[bv*4+cb, bv] = 1
    s4 = np.zeros((128, NBV), dtype=np.float32)
    for bv in range(NBV):
        s4[bv * 4:(bv + 1) * 4, bv] = 1.0
    sh["S4"] = s4.astype(BF)
    sh["I128"] = np.eye(128, dtype=np.float32).astype(BF)
    fcw = np.zeros((128, 7 * OUT), dtype=np.float32)
    for k in range(7):
        fcw[:, k * OUT:(k + 1) * OUT] = f["fc_w"][k * D:(k + 1) * D, :]
    sh["fcw"] = fcw.astype(BF)
    sh["fcb2"] = np.tile(f["fc_b"][None, :], (BL, 1)).astype(np.float32)
    return sh


def _mon_flat(tok):
    """tok [BL,V,M,L] int -> flat [NCHUNK*TOKC] with order
    flat[c*TOKC + (mi*NBV + b*V + v)*L + l] = tok[b, v, c*TCH+mi, l]."""
    t = np.asarray(tok).reshape(BL, V, NCHUNK, TCH, L)
    return np.ascontiguousarray(t.transpose(2, 3, 0, 1, 4)).reshape(-1)


def _prep_core(inputs, shared, core):
    b0 = core * BL
    m = dict(shared)
    # item gather indices
    for name in ("lab_item",):
        flat = _mon_flat(np.asarray(inputs["tok_" + name])[b0:b0 + BL])
        m["idx_" + name] = _wrap_idx(flat)
    # one-hot matrices [128, NCHUNK*nvc*TOKC] fp16
    for name, tag, nvc in (("lab_value", "oh_lab", 2),
                           ("inj_value", "oh_inj", 2),
                           ("inj_item", "oh_inji", 4)):
        flat = _mon_flat(np.asarray(inputs["tok_" + name])[b0:b0 + BL])
        toks = flat.reshape(NCHUNK, TOKC)
        oh = np.zeros((128, NCHUNK, nvc, TOKC), dtype=BF)
        prow = np.arange(128)[:, None]
        for c in range(NCHUNK):
            for vc in range(nvc):
                oh[:, c, vc, :] = (toks[c][None, :] == vc * 128 + prow)
        m[tag] = np.ascontiguousarray(oh).reshape(128, NCHUNK * nvc * TOKC)
    # visit-stream indices: flat[r*128 + (b*V+v)*4 + cb] = tok[b,v,cb*6+r]
    for name in ("cond", "proc", "drug"):
        t = np.asarray(inputs["tok_" + name])[b0:b0 + BL].reshape(BL, V, 4, 6)
        flat = np.ascontiguousarray(t.transpose(3, 0, 1, 2)).reshape(-1)
        m["idx_" + name] = _wrap_idx(flat)
    wa = np.zeros((1, 64), dtype=np.float32)
    wa[0, :NBV] = np.asarray(inputs["weight"], np.float32)[b0:b0 + BL].reshape(NBV)
    wa[0, NBV:] = np.asarray(inputs["age"], np.float32)[b0:b0 + BL].reshape(NBV)
    m["wa"] = wa.astype(BF)
    return m


# --------------------------------------------------------------------------
# device program
# --------------------------------------------------------------------------

CONSTS = (("mwhhT", [128, 1920], BF16), ("mwihT012", [128, 1152], BF16),
          ("mwihT34", [128, 768], BF16), ("vwhhT", [128, 2688], BF16),
          ("vwihT04", [128, 1920], BF16), ("mb012c", [128, 9], F32),
          ("mb34c", [128, 6], F32), ("vb04c", [128, 15], F32),
          ("mbhn_bc", [128, 160], F32), ("vbhn_bc", [128, 14], F32),
          ("vxg56u", [1, 768], BF16), ("vb56c", [128, 6], F32),
          ("S4", [128, NBV], BF16), ("I128", [128, 128], BF16),
          ("fcw", [128, 7 * OUT], BF16), ("fcb2", [BL, OUT], F32),
          ("wa", [1, 64], BF16), ("vt_lab", [128, 256], BF16),
          ("vt_inj", [128, 256], BF16), ("vt_inji", [128, 512], BF16))


def build_nc(debug=False):
    nc = bacc.Bacc("TRN2", target_bir_lowering=False, debug=False,
                   num_devices=NCORES)
    dt = {}
    for name in ("cond", "proc", "drug", "lab_item"):
        dt["emb_" + name] = nc.dram_tensor("emb_" + name, [VOCAB[name], D],
                                           BF16, kind="ExternalInput")
    for name in ("lab_item",):
        dt["idx_" + name] = nc.dram_tensor("idx_" + name,
                                           [128, NCHUNK * TOKC // 16], I16,
                                           kind="ExternalInput")
    for name in ("cond", "proc", "drug"):
        dt["idx_" + name] = nc.dram_tensor("idx_" + name, [128, VIDX // 16],
                                           I16, kind="ExternalInput")
    for name, nvc in (("oh_lab", 2), ("oh_inj", 2), ("oh_inji", 4)):
        dt[name] = nc.dram_tensor(name, [128, NCHUNK * nvc * TOKC], BF16,
                                  kind="ExternalInput")
    for name, shape, dty in CONSTS:
        dt[name] = nc.dram_tensor(name, shape, dty, kind="ExternalInput")
    out_logits = nc.dram_tensor("logits", [BL, OUT], F32, kind="ExternalOutput")
    if debug:
        dt["dbg_eT"] = nc.dram_tensor("dbg_eT", [128, NBV], F32,
                                      kind="ExternalOutput")
        dt["dbg_labT"] = nc.dram_tensor("dbg_labT", [128, GC], F32,
                                        kind="ExternalOutput")
        dt["dbg_h1"] = nc.dram_tensor("dbg_h1", [128, 160], F32,
                                      kind="ExternalOutput")
        dt["dbg_hfin"] = nc.dram_tensor("dbg_hfin", [128, 160], F32,
                                        kind="ExternalOutput")
        dt["dbg_vxg"] = nc.dram_tensor("dbg_vxg", [128, 672], F32,
                                       kind="ExternalOutput")
        dt["dbg_xgc"] = nc.dram_tensor("dbg_xgc", [128, 288], F32,
                                       kind="ExternalOutput")

    with tile.TileContext(nc) as tc:
        _program(nc, tc, dt, out_logits, debug)
    nc.compile()
    return nc


def _program(nc, tc, dt, out_logits, debug=False):
    import contextlib
    ctx = contextlib.ExitStack()
    with ctx:
        cpool = ctx.enter_context(tc.tile_pool(name="const", bufs=1))
        vgpool = ctx.enter_context(tc.tile_pool(name="vgather", bufs=1))
        gpool = ctx.enter_context(tc.tile_pool(name="gather", bufs=2))
        ohpool = ctx.enter_context(tc.tile_pool(name="oh", bufs=2))
        spool = ctx.enter_context(tc.tile_pool(name="work", bufs=2))
        xgpool = ctx.enter_context(tc.tile_pool(name="xg", bufs=2))
        hpool = ctx.enter_context(tc.tile_pool(name="h", bufs=2))
        ppool = ctx.enter_context(tc.tile_pool(name="psum", bufs=2,
                                               space="PSUM"))

        cb = {}
        for name, shape, dty in CONSTS:
            t = cpool.tile(shape, dty, tag=name, name=name)
            nc.sync.dma_start(t[:], dt[name].ap())
            cb[name] = t
        idx = {}
        for name in ("cond", "proc", "drug", "lab_item"):
            shape = [128, VIDX // 16] if name in ("cond", "proc", "drug") \
                else [128, NCHUNK * TOKC // 16]
            t = cpool.tile(shape, I16, tag="idx_" + name, name="idx_" + name)
            nc.sync.dma_start(t[:], dt["idx_" + name].ap())
            idx[name] = t

        I128 = cb["I128"]

        # ---- visit-level features eT[k] = [128 d, 32 bv] (bf16)
        eTb = {}
        for name in ("cond", "proc", "drug"):
            vG = vgpool.tile([128, VIDX], BF16, tag="vG_" + name, name="vG")
            nc.gpsimd.dma_gather(
                vG[:].rearrange("p (r e) -> p r e", e=D),
                dt["emb_" + name].ap(), idx[name][:], VIDX, VIDX, D)
            eTp = ppool.tile([128, NBV], F32, tag="pxg", name="eTp")
            for r in range(6):
                nc.tensor.matmul(eTp[:],
                                 vG[:].rearrange("p (r e) -> p r e", e=D)[:, r, :],
                                 cb["S4"][:], start=(r == 0), stop=(r == 5))
            et = cpool.tile([128, NBV], BF16, tag="eT_" + name, name="eT")
            nc.scalar.copy(et[:], eTp[:])
            if debug and name == "proc":
                etf = cpool.tile([128, NBV], F32, tag="etf", name="etf")
                nc.scalar.copy(etf[:], eTp[:])
                nc.sync.dma_start(dt["dbg_eT"].ap(), etf[:])
            eTb[name] = et

        # ---- xgc for monitor keys 0-2: [128, 3, 96] (gi, k*32+bv), bias folded
        xgc = cpool.tile([128, 3, 96], F32, tag="xgc", name="xgc")
        for k, name in enumerate(("cond", "proc", "drug")):
            pk = ppool.tile([128, 96], F32, tag="pxg", name="pk")
            for gi in range(3):
                nc.tensor.matmul(pk[:, gi * 32:(gi + 1) * 32],
                                 cb["mwihT012"][:, (k * 3 + gi) * 128:
                                                (k * 3 + gi + 1) * 128],
                                 eTb[name][:], start=True, stop=True)
            for gi in range(3):
                nc.scalar.activation(xgc[:, gi, k * 32:(k + 1) * 32],
                                     pk[:, gi * 32:(gi + 1) * 32], AF.Identity,
                                     bias=cb["mb012c"][:, k * 3 + gi:
                                                       k * 3 + gi + 1])

        if debug:
            nc.sync.dma_start(dt["dbg_xgc"].ap(),
                              xgc[:].rearrange("p a b -> p (a b)"))

        # ---- visit xg for keys 5,6 (weight/age), rank-1 + const
        vxg = cpool.tile([128, 3, 7, NBV], F32, tag="vxg", name="vxg")
        for k in (5, 6):
            p56 = ppool.tile([128, 96], F32, tag="pxg", name="p56")
            for gi in range(3):
                nc.tensor.matmul(p56[:, gi * 32:(gi + 1) * 32],
                                 cb["vxg56u"][0:1, ((k - 5) * 3 + gi) * 128:
                                              ((k - 5) * 3 + gi + 1) * 128],
                                 cb["wa"][0:1, (k - 5) * 32:(k - 4) * 32],
                                 start=True, stop=True)
            for gi in range(3):
                nc.scalar.activation(vxg[:, gi, k, :],
                                     p56[:, gi * 32:(gi + 1) * 32], AF.Identity,
                                     bias=cb["vb56c"][:, (k - 5) * 3 + gi:
                                                      (k - 5) * 3 + gi + 1])

        # ---- monitor chunks + GRU chain
        h = hpool.tile([128, 160], BF16, tag="h", name="h0")
        nc.vector.memset(h[:], 0.0)

        for c in range(NCHUNK):
            # lab_item: gather + per-rank PE transpose into [d, tok]
            iG = gpool.tile([128, TOKC], BF16, tag="iG", name="iG")
            iGr = iG[:].rearrange("p (r e) -> p r e", e=D)
            for s in range(3):
                nc.gpsimd.dma_gather(
                    iGr[:, s * 8:(s + 1) * 8, :],
                    dt["emb_lab_item"].ap(),
                    idx["lab_item"][:, c * (TOKC // 16) + s * 64:
                                    c * (TOKC // 16) + (s + 1) * 64],
                    1024, 1024, D)
            ohs = {}
            for name, nvc in (("oh_lab", 2), ("oh_inj", 2), ("oh_inji", 4)):
                t = ohpool.tile([128, nvc, TOKC], BF16, tag=name, name="ohT")
                nc.sync.dma_start(
                    t[:], dt[name].ap()
                    .rearrange("p (c x) -> p c x", c=NCHUNK)[:, c, :]
                    .rearrange("p (v t) -> p v t", v=nvc))
                ohs[name] = t
            prod3 = spool.tile([128, TOKC], BF16, tag="prod3", name="prod3")
            prod4 = spool.tile([128, TOKC], BF16, tag="prod4", name="prod4")
            for blk in range(6):
                sl = slice(blk * 512, (blk + 1) * 512)
                # lab: transpose item ranks; value via one-hot matmul
                tP = ppool.tile([128, 512], BF16, tag="tP", name="tP")
                for rr in range(4):
                    nc.tensor.transpose(tP[:, rr * 128:(rr + 1) * 128],
                                        iGr[:, blk * 4 + rr, :], I128[:])
                valP = ppool.tile([128, 512], F32, tag="valP", name="valP")
                for vc in range(2):
                    nc.tensor.matmul(valP[:],
                                     cb["vt_lab"][:, vc * 128:(vc + 1) * 128],
                                     ohs["oh_lab"][:, vc, sl],
                                     start=(vc == 0), stop=(vc == 1))
                valS = spool.tile([128, 512], BF16, tag="valS", name="valS")
                if blk % 2 == 0:
                    nc.scalar.copy(valS[:], valP[:])
                else:
                    nc.vector.tensor_copy(valS[:], valP[:])
                nc.vector.tensor_tensor(prod3[:, sl], tP[:], valS[:],
                                        op=mybir.AluOpType.mult)
                # inj: both streams via one-hot matmuls
                valPi = ppool.tile([128, 512], F32, tag="valP", name="valPi")
                for vc in range(2):
                    nc.tensor.matmul(valPi[:],
                                     cb["vt_inj"][:, vc * 128:(vc + 1) * 128],
                                     ohs["oh_inj"][:, vc, sl],
                                     start=(vc == 0), stop=(vc == 1))
                valSi = spool.tile([128, 512], BF16, tag="valSi", name="valSi")
                if blk % 2 == 0:
                    nc.vector.tensor_copy(valSi[:], valPi[:])
                else:
                    nc.scalar.copy(valSi[:], valPi[:])
                itemP = ppool.tile([128, 512], F32, tag="valP", name="itemP")
                for vc in range(4):
                    nc.tensor.matmul(itemP[:],
                                     cb["vt_inji"][:, vc * 128:(vc + 1) * 128],
                                     ohs["oh_inji"][:, vc, sl],
                                     start=(vc == 0), stop=(vc == 3))
                nc.vector.tensor_tensor(prod4[:, sl], itemP[:], valSi[:],
                                        op=mybir.AluOpType.mult)
            xg34c = xgpool.tile([128, 3, TCH, 64], F32, tag="xg34c",
                                name="xg34c")
            for k, prodX in ((3, prod3), (4, prod4)):
                red = spool.tile([128, GC], F32, tag=f"red{k}", name="red")
                nc.vector.tensor_reduce(
                    red[:], prodX[:].rearrange("p (g l) -> p g l", l=L),
                    axis=mybir.AxisListType.X, op=mybir.AluOpType.add)
                redb = spool.tile([128, GC], BF16, tag=f"redb{k}", name="redb")
                nc.scalar.copy(redb[:], red[:])
                pxg = ppool.tile([128, 384], F32, tag="pxg", name="pxg")
                for gi in range(3):
                    nc.tensor.matmul(pxg[:, gi * 128:(gi + 1) * 128],
                                     cb["mwihT34"][:, ((k - 3) * 3 + gi) * 128:
                                                   ((k - 3) * 3 + gi + 1) * 128],
                                     redb[:], start=True, stop=True)
                for gi in range(3):
                    nc.scalar.activation(
                        xg34c[:, gi, :, (k - 3) * 32:(k - 2) * 32],
                        pxg[:, gi * 128:(gi + 1) * 128]
                        .rearrange("p (m b) -> p m b", m=TCH), AF.Identity,
                        bias=cb["mb34c"][:, (k - 3) * 3 + gi:
                                         (k - 3) * 3 + gi + 1])
                if debug and c == 0 and k == 3:
                    nc.sync.dma_start(dt["dbg_labT"].ap(), red[:])

            for mi in range(TCH):
                przt = ppool.tile([128, 320], F32, tag="prz", name="przt",
                                  bufs=1)
                prz = przt[:].rearrange("p (g x) -> p g x", g=2)
                pn = ppool.tile([128, 160], F32, tag="pn", name="pn", bufs=1)
                nc.scalar.copy(prz[:, :, 0:96], xgc[:, 0:2, :])
                nc.scalar.copy(prz[:, :, 96:160], xg34c[:, 0:2, mi, :])
                for k in range(5):
                    hs = h[:, k * 32:(k + 1) * 32]
                    for gi in range(2):
                        nc.tensor.matmul(
                            prz[:, gi, k * 32:(k + 1) * 32],
                            cb["mwhhT"][:, (k * 3 + gi) * 128:
                                        (k * 3 + gi + 1) * 128],
                            hs, start=False, stop=True, skip_group_check=True)
                    nc.tensor.matmul(
                        pn[:, k * 32:(k + 1) * 32],
                        cb["mwhhT"][:, (k * 3 + 2) * 128:(k * 3 + 3) * 128],
                        hs, start=True, stop=True)
                r = spool.tile([128, 160], F32, tag="r", name="r")
                nc.scalar.activation(r[:], przt[:, 0:160], AF.Sigmoid)
                z = spool.tile([128, 160], BF16, tag="z", name="z")
                nc.scalar.activation(z[:], przt[:, 160:320], AF.Sigmoid)
                u = spool.tile([128, 160], F32, tag="u", name="u")
                nc.vector.tensor_tensor(u[:], pn[:], cb["mbhn_bc"][:],
                                        op=mybir.AluOpType.add)
                nc.vector.tensor_tensor(u[:], r[:], u[:],
                                        op=mybir.AluOpType.mult)
                npre = spool.tile([128, 160], F32, tag="npre", name="npre")
                nc.vector.tensor_tensor(npre[:, 0:96], u[:, 0:96],
                                        xgc[:, 2, :], op=mybir.AluOpType.add)
                nc.vector.tensor_tensor(npre[:, 96:160], u[:, 96:160],
                                        xg34c[:, 2, mi, :],
                                        op=mybir.AluOpType.add)
                nt = spool.tile([128, 160], BF16, tag="nt", name="nt")
                nc.scalar.activation(nt[:], npre[:], AF.Tanh)
                t3 = spool.tile([128, 160], BF16, tag="t3", name="t3")
                nc.vector.tensor_tensor(t3[:], h[:], nt[:],
                                        op=mybir.AluOpType.subtract)
                nc.vector.tensor_tensor(t3[:], t3[:], z[:],
                                        op=mybir.AluOpType.mult)
                h = hpool.tile([128, 160], BF16, tag="h", name="h")
                nc.vector.tensor_tensor(h[:], t3[:], nt[:],
                                        op=mybir.AluOpType.add)
                if debug and c == 0 and mi == 0:
                    hf = spool.tile([128, 160], F32, tag="hf", name="hf")
                    nc.vector.tensor_copy(hf[:], h[:])
                    nc.sync.dma_start(dt["dbg_h1"].ap(), hf[:])

        if debug:
            hf2 = spool.tile([128, 160], F32, tag="hf", name="hf2")
            nc.vector.tensor_copy(hf2[:], h[:])
            nc.sync.dma_start(dt["dbg_hfin"].ap(), hf2[:])

        # ---- visit xg for keys 0-4 from monitor hidden state
        for k in range(5):
            pk = ppool.tile([128, 96], F32, tag="pxg", name="pkv")
            for gi in range(3):
                nc.tensor.matmul(pk[:, gi * 32:(gi + 1) * 32],
                                 cb["vwihT04"][:, (k * 3 + gi) * 128:
                                               (k * 3 + gi + 1) * 128],
                                 h[:, k * 32:(k + 1) * 32],
                                 start=True, stop=True)
            for gi in range(3):
                nc.scalar.activation(vxg[:, gi, k, :],
                                     pk[:, gi * 32:(gi + 1) * 32], AF.Identity,
                                     bias=cb["vb04c"][:, k * 3 + gi:
                                                      k * 3 + gi + 1])

        if debug:
            nc.sync.dma_start(dt["dbg_vxg"].ap(),
                              vxg[:].rearrange("p a b c2 -> p (a b c2)"))

        # ---- visit GRU chain (7 keys, 16 steps, batch BL=2 per key)
        vxgv = vxg[:].rearrange("p g k (b v2) -> p g k b v2", b=BL)
        vh = hpool.tile([128, 14], BF16, tag="vh", name="vh0")
        nc.vector.memset(vh[:], 0.0)
        for v in range(V):
            pvrzt = ppool.tile([128, 28], F32, tag="prz", name="pvrzt",
                               bufs=1)
            pvrz = pvrzt[:].rearrange("p (g x) -> p g x", g=2)
            pvn = ppool.tile([128, 14], F32, tag="pn", name="pvn", bufs=1)
            nc.scalar.copy(pvrz[:, :, :], vxgv[:, 0:2, :, :, v])
            for k in range(7):
                hs = vh[:, k * 2:(k + 1) * 2]
                for gi in range(2):
                    nc.tensor.matmul(
                        pvrz[:, gi, k * 2:(k + 1) * 2],
                        cb["vwhhT"][:, (k * 3 + gi) * 128:
                                    (k * 3 + gi + 1) * 128],
                        hs, start=False, stop=True, skip_group_check=True)
                nc.tensor.matmul(
                    pvn[:, k * 2:(k + 1) * 2],
                    cb["vwhhT"][:, (k * 3 + 2) * 128:(k * 3 + 3) * 128],
                    hs, start=True, stop=True)
            vr = spool.tile([128, 14], F32, tag="vr", name="vr")
            nc.scalar.activation(vr[:], pvrzt[:, 0:14], AF.Sigmoid)
            vz = spool.tile([128, 14], BF16, tag="vz", name="vz")
            nc.scalar.activation(vz[:], pvrzt[:, 14:28], AF.Sigmoid)
            vu = spool.tile([128, 14], F32, tag="vu", name="vu")
            nc.vector.tensor_tensor(vu[:], pvn[:], cb["vbhn_bc"][:],
                                    op=mybir.AluOpType.add)
            nc.vector.tensor_tensor(vu[:], vr[:], vu[:],
                                    op=mybir.AluOpType.mult)
            nc.vector.tensor_tensor(vu[:], vu[:], vxgv[:, 2, :, :, v],
                                    op=mybir.AluOpType.add)
            vnt = spool.tile([128, 14], BF16, tag="vnt", name="vnt")
            nc.scalar.activation(vnt[:], vu[:], AF.Tanh)
            vt3 = spool.tile([128, 14], BF16, tag="vt3", name="vt3")
            nc.vector.tensor_tensor(vt3[:], vh[:], vnt[:],
                                    op=mybir.AluOpType.subtract)
            nc.vector.tensor_tensor(vt3[:], vt3[:], vz[:],
                                    op=mybir.AluOpType.mult)
            vh = hpool.tile([128, 14], BF16, tag="vh", name="vh")
            nc.vector.tensor_tensor(vh[:], vt3[:], vnt[:],
                                    op=mybir.AluOpType.add)

        # ---- FC head
        rlb = spool.tile([128, 14], BF16, tag="rlb", name="rlb")
        nc.scalar.activation(rlb[:], vh[:], AF.Relu)
        pfc = ppool.tile([BL, OUT], F32, tag="pn", name="pfc", bufs=1)
        for k in range(7):
            nc.tensor.matmul(pfc[:], rlb[:, k * 2:(k + 1) * 2],
                             cb["fcw"][:, k * OUT:(k + 1) * OUT],
                             start=(k == 0), stop=(k == 6))
        lg = spool.tile([BL, OUT], F32, tag="lg", name="lg")
        nc.vector.tensor_tensor(lg[:], pfc[:], cb["fcb2"][:],
                                op=mybir.AluOpType.add)
        nc.sync.dma_start(out_logits.ap(), lg[:])


# --------------------------------------------------------------------------
# entry point
# --------------------------------------------------------------------------

_NC_CACHE = None


def kernel(**inputs):
    global _NC_CACHE
    if _NC_CACHE is None:
        _NC_CACHE = build_nc()
    nc = _NC_CACHE
    shared = _prep_shared(inputs)
    in_maps = [_prep_core(inputs, shared, c) for c in range(NCORES)]
    res = run_bass_kernel_spmd(nc, in_maps, core_ids=list(range(NCORES)))
    return np.concatenate([res.results[c]["logits"] for c in range(NCORES)],
                          axis=0).astype(np.float32)


if __name__ == "__main__":
    import reference
    inputs = {k: np.asarray(v) for k, v in reference.setup_inputs().items()}
    out = kernel(**inputs)
    print("out", out.shape, out.dtype)
